# revision 52
# baseline (speedup 1.0000x reference)
"""Trainium2 Bass kernel for nn_Attn (additive/Bahdanau-style attention).

Math (per batch b):
    Wh, We   = W[:, :D], W[:, D:]                       # [D,D] each
    energy   = tanh(enc @ We.T + hidden @ Wh.T + b)     # [S, D]
    scores   = energy @ v, masked to length, softmax    # [S]
    context  = scores @ enc                             # [D]

Sharding / packing: data-parallel over batch B=16 across 8 cores,
length-aware.  Each core takes a (short, long) pair (longest batch with
shortest); the SHORT batch's chunks are packed first, the long one
follows at 128-chunk granularity (no tile padding) -- NF = max flat
128-chunks over cores.  The first 4 chunks (s-tile 0) are computed in
BF16 ("repair zone"): fp8 score noise is amplified ~1/sqrt(len) by the
softmax, so short batches (which sit in the zone) get full precision
while long ones tolerate fp8.  All per-core structure (ownership,
validity, per-chunk tanh bias) rides host-prepared data: the hid/b bias
(hidden @ Wh^T + b, a trivial host matmul) is shipped per flat chunk
(biasC), so mixed-ownership tiles need no program branches.

Device-side structure:
  - pass 1 computes energy^T tiles [e=128, s<=512]: s-tile 0 with
    stationary-We^T bf16 matmuls, the rest in fp8e4 with DoubleRow
    perf mode (K=256 per instruction: adjacent 128-chunk pairs of the
    contraction ride the two slots) at 2x the bf16 rate.
  - tanh ACTs take the per-chunk host bias; chunks below the mixed-
    ownership watermark (mz) get per-chunk calls, uniform tiles one call.
  - the v-dot accumulates on the DVE in bf16 (2x rate); the partition
    reduce is a single bf16 matmul per chunk (fp32 would split into two
    half-rate passes on HW).  exp uses NO shift: |score| <= sum|v| ~ 25
    cannot overflow fp32, and small arguments keep full precision.
  - pass 2 accumulates both batch contexts at once into two [2, 512]
    PSUM halves with [s=128, 2] masked bf16 weight columns; 1/sum folds
    into the output scale on two engines with parallel store DMAs.
  - tile order: [last (smallest: ramps the PE p-state while DMA
    streams)], [0 (bf16)], middle fp8 tiles in 3s, [nt-2] last so the
    non-overlapped tail chain covers one tile.  Each group's softmax/
    pass-2 work is deferred into the next group's ec loop (ec 3/5/6) so
    the PE always has matmul runway ahead of the dependencies.
"""

import numpy as np

B, S, D = 16, 2048, 1024
NCORES = 8
BL = B // NCORES   # batches per core
ST = 512           # s-tile width (pass-1 moving dim; one PSUM bank)
DC = D // 128      # contraction / e chunks
NPT = ST // 128    # 128-wide flat chunks per full s-tile
REP0 = 2           # chunks in the bf16 repair tile (s-tile 0)

_NC_CACHE = {}


def _build_program(nt, nf, mz, mzlo, stage="all"):
    import concourse.bacc as bacc
    import concourse.bass as bass
    import concourse.mybir as mybir
    import concourse.tile as tile

    f32 = mybir.dt.float32
    bf16 = mybir.dt.bfloat16
    f8 = mybir.dt.float8e4
    DoubleRow = mybir.MatmulPerfMode.DoubleRow
    Tanh = mybir.ActivationFunctionType.Tanh
    Exp = mybir.ActivationFunctionType.Exp
    Alu = mybir.AluOpType

    d = D

    def c0(t):
        # first flat chunk of tile t (tile 0 holds REP0 chunks)
        return 0 if t == 0 else REP0 + NPT * (t - 1)

    def nch(t):
        # chunks in tile t (the last tile may be partial)
        return min(REP0 if t == 0 else NPT, nf - c0(t))

    # processing order (see module docstring): small tile first (ramps the
    # PE on tiny DMA), then the other fp8 tiles (small DMA footprint)
    # while the bf16 tile's 3 MiB streams in, the bf16 repair tile, and
    # tile nt-2 as the tail group.
    tlast = nt - 1
    groups = [[tlast]]
    mid = list(range(1, nt - 2))
    while mid:
        groups.append(mid[:3])
        mid = mid[3:]
    groups.append([nt - 2])
    groups.append([0])   # smallest tile last: shortest tail chain
    # pass-2 / encf consumption order = group emission order
    chunk_order = []
    for g in groups:
        for t_ in g:
            chunk_order.extend(range(c0(t_), c0(t_) + nch(t_)))

    nc = bacc.Bacc()
    # all big inputs are host-prearranged partition-major so every DMA is
    # a straight [128, X] copy.  Weights are split into per-ec stripes so
    # pass-1 can start as soon as stripe 0 lands.
    etb_d = nc.declare_dram_parameter("etb", [128, DC, REP0 * 128], bf16, isOutput=False)
    # per-tile slabs with each tile's [DC, w] packed dense so partial
    # tiles still DMA as one contiguous DC*w line per partition
    et8_d = nc.declare_dram_parameter("et8", [nt - 1, 128, DC * ST], f8, isOutput=False)
    encf_d = nc.declare_dram_parameter("encf", [128, nf, d], bf16, isOutput=False)
    # weight stripes partition-major so multi-stripe DMAs are single
    # contiguous-line descriptors
    weTsB_d = nc.declare_dram_parameter("weTsB", [128, DC, DC, 128], bf16, isOutput=False)
    weTs8_d = nc.declare_dram_parameter("weTs8", [128, DC, DC, 128], f8, isOutput=False)
    biasC_d = nc.declare_dram_parameter("biasC", [128, DC, nf], f32, isOutput=False)
    vcol_d = nc.declare_dram_parameter("vcol", [128, DC], f32, isOutput=False)
    vcolb_d = nc.declare_dram_parameter("vcolb", [128, DC], bf16, isOutput=False)
    pmo_d = nc.declare_dram_parameter("pmo", [128, 3, nf], f32, isOutput=False)
    if stage == "all":
        out_d = nc.declare_dram_parameter("ctx_out", [BL, d], f32, isOutput=True)
    else:
        out_d = nc.declare_dram_parameter("ctx_out", [128, nf], f32, isOutput=True)

    with tile.TileContext(nc) as tc:
        with (
            tc.tile_pool(name="consts", bufs=1) as consts,
            tc.tile_pool(name="etp", bufs=1) as etp,
            tc.tile_pool(name="enf", bufs=1) as enf,
            tc.tile_pool(name="enp", bufs=4) as enp,
            tc.tile_pool(name="psA", bufs=4, space="PSUM") as psA,
            tc.tile_pool(name="psS", bufs=2, space="PSUM") as psS,
            tc.tile_pool(name="psM", bufs=2, space="PSUM") as psM,
        ):
            # ---------------- DMA emission --------------------------------
            # one ordered stream on the sync queue: the tail tile's enc +
            # fp8 weight stripes (group 1 runs on them immediately), the
            # bf16 tile + its stripes, the remaining fp8 tiles, then encf
            # in pass-2 consumption order.  Tiny consts ride gpsimd.
            weTsB_sb = consts.tile([128, DC, DC, 128], bf16, name="weTsB")
            weTs8_sb = consts.tile([128, DC, DC, 128], f8, name="weTs8")
            etb_sb = etp.tile([128, DC, REP0 * 128], bf16, name="etb")
            et8_sb = etp.tile([128, nt - 1, DC, ST], f8, name="et8")
            en2_sb = enf.tile([128, nf, d], bf16, name="en2")

            def dma_et8(t_, q=None):
                w_ = nch(t_) * 128
                (q or nc.sync).dma_start(
                    out=et8_sb[:, t_ - 1, :, 0:w_],
                    in_=et8_d[t_ - 1][:, 0:DC * w_],
                )

            def dma_encf(c0, c1):
                nc.sync.dma_start(
                    out=en2_sb[:, c0:c1, :], in_=encf_d[:, c0:c1, :]
                )

            # DMA: the first group's tile and etb ride the scalar queue
            # (the second hardware DGE, idle in the prefix) in parallel
            # with the weight stripes on sync; tiny consts go software-DGE
            # on gpsimd; everything else is ordered on sync just ahead of
            # its consumer.
            wl = nch(tlast) * 128
            nc.scalar.dma_start(
                out=et8_sb[:, tlast - 1, 0:4, 0:wl], in_=et8_d[tlast - 1][:, 0:4 * wl]
            )
            nc.scalar.dma_start(
                out=et8_sb[:, tlast - 1, 4:DC, 0:wl],
                in_=et8_d[tlast - 1][:, 4 * wl:DC * wl],
            )
            nc.scalar.dma_start(out=etb_sb, in_=etb_d[:, :, :])
            biasC_sb = consts.tile([128, DC, nf], f32)
            nc.gpsimd.dma_start(out=biasC_sb, in_=biasC_d[:, :, :])
            vcol_sb = consts.tile([128, DC], f32)
            nc.gpsimd.dma_start(out=vcol_sb, in_=vcol_d[:, :])
            vcolb_sb = consts.tile([128, DC], bf16)
            nc.gpsimd.dma_start(out=vcolb_sb, in_=vcolb_d[:, :])
            pmo_sb = consts.tile([128, 3, nf], f32)
            nc.gpsimd.dma_start(out=pmo_sb, in_=pmo_d[:, :, :])
            posf_sb = pmo_sb[:, 0, :]
            lenmap_sb = pmo_sb[:, 1, :]
            own0_sb = pmo_sb[:, 2, :]

            nc.sync.dma_start(out=weTs8_sb[:, 0], in_=weTs8_d[:, 0])
            nc.sync.dma_start(out=weTs8_sb[:, 1:4], in_=weTs8_d[:, 1:4])
            nc.sync.dma_start(out=weTs8_sb[:, 4:DC], in_=weTs8_d[:, 4:DC])
            for t_ in range(1, nt - 2):
                dma_et8(t_)
            dma_encf(c0(tlast), c0(tlast) + nch(tlast))
            nc.sync.dma_start(out=weTsB_sb[:, 0:4], in_=weTsB_d[:, 0:4])
            dma_encf(c0(1), c0(2))
            nc.sync.dma_start(out=weTsB_sb[:, 4:DC], in_=weTsB_d[:, 4:DC])
            dma_et8(nt - 2)
            if nt > 4:
                dma_encf(c0(2), c0(nt - 2))
            dma_encf(c0(nt - 2), c0(nt - 1))
            dma_encf(0, REP0)

            # PE warmup: dummy matmuls on memset data ramp the tensor
            # engine's p-state during the launch/DMA-prefix window.
            warm = consts.tile([128, 512], bf16)
            nc.vector.memset(warm, 0.0)
            for i in range(12):
                pw = psA.tile([128, 512], f32, tag="proj", name=f"warm{i}")
                nc.tensor.matmul(pw, warm[:, 0:128], warm, start=True, stop=True)

            # ---------------- small constants -----------------------------
            onesb = consts.tile([128, 1], bf16)
            nc.vector.memset(onesb, 1.0)
            ones32 = consts.tile([128, 1], f32)
            nc.vector.memset(ones32, 1.0)

            # masks from host-relayout index tensors: valid = pos < len,
            # then split by batch-slot ownership (slot 0 = short batch).
            valid_sb = consts.tile([128, nf], f32)
            nc.vector.scalar_tensor_tensor(
                valid_sb, posf_sb, 1.0, lenmap_sb, op0=Alu.mult, op1=Alu.is_lt
            )
            mask0 = consts.tile([128, nf], f32)
            nc.vector.scalar_tensor_tensor(
                mask0, valid_sb, 1.0, own0_sb, op0=Alu.mult, op1=Alu.mult
            )
            mask1 = consts.tile([128, nf], f32)
            nc.vector.scalar_tensor_tensor(
                mask1, valid_sb, 1.0, mask0, op0=Alu.mult, op1=Alu.subtract
            )

            # ---------------- pass 1 + pipelined softmax / pass 2 ---------
            scores_sb = consts.tile([128, nf], f32)
            exp_sb = consts.tile([128, nf], f32)
            attn2b = consts.tile([128, nf, 2], bf16)
            mexp0 = consts.tile([128, nf], f32)
            mexp1 = consts.tile([128, nf], f32)
            psums01 = consts.tile([128, 2], f32)
            cps = [
                psM.tile([BL, 512], f32, tag="m", name="cps0"),
                psM.tile([BL, 512], f32, tag="m", name="cps1"),
            ]

            pend = None            # (tiles, accs, en7) of the previous group
            pend_sps = None
            p2_done = 0            # chunks whose pass-2 mms are emitted

            def tanh_emit(out, ps, ec, t_, w_):
                # ownership varies per core only in chunks [mzlo, mz): those
                # get per-chunk bias ACTs; runs outside are merged calls.
                cb = c0(t_)
                j = 0
                while j < nch(t_):
                    cj = cb + j
                    if cj < mzlo:
                        je = min(nch(t_), mzlo - cb)
                    elif cj >= mz:
                        je = nch(t_)
                    else:
                        je = j + 1
                    nc.scalar.activation(
                        out[:, j * 128:je * 128], ps[:, j * 128:je * 128], Tanh,
                        bias=biasC_sb[:, ec, cj:cj + 1],
                    )
                    j = je

            def emit_reduces(tiles, accs, en7):
                # partition-reduce each bf16 acc column block into one PSUM
                # tile (separate cols).  For the tail group (en7 set), the
                # final ec's v-dot arrives as a second accumulated matmul.
                sps = psS.tile([128, sum(nch(t_) for t_ in tiles)], f32, tag="s")
                off = 0
                for t_ in tiles:
                    for c in range(nch(t_)):
                        nc.tensor.matmul(
                            sps[:, off:off + 1],
                            accs[t_][:, c * 128:(c + 1) * 128],
                            onesb[:, 0:1],
                            start=True,
                            stop=(en7 is None),
                        )
                        if en7 is not None:
                            nc.tensor.matmul(
                                sps[:, off:off + 1],
                                en7[t_][:, c * 128:(c + 1) * 128],
                                vcolb_sb[:, DC - 1:DC],
                                start=False,
                                stop=True,
                            )
                        off += 1
                if stage == "p1":
                    f0 = c0(tiles[0])
                    f1 = c0(tiles[-1]) + nch(tiles[-1])
                    nc.vector.tensor_copy(scores_sb[:, f0:f1], sps)
                return sps

            def emit_softmax(tiles, sps):
                # no shift: |score| <= sum|v| ~ 25 cannot overflow fp32,
                # and small arguments keep the exp table's full precision.
                f0 = c0(tiles[0])
                f1 = c0(tiles[-1]) + nch(tiles[-1])
                nc.scalar.activation(exp_sb[:, f0:f1], sps, Exp)
                nc.vector.scalar_tensor_tensor(
                    attn2b[:, f0:f1, 0], exp_sb[:, f0:f1], 1.0, mask0[:, f0:f1],
                    op0=Alu.mult, op1=Alu.mult,
                )
                nc.vector.scalar_tensor_tensor(
                    attn2b[:, f0:f1, 1], exp_sb[:, f0:f1], 1.0, mask1[:, f0:f1],
                    op0=Alu.mult, op1=Alu.mult,
                )

            def emit_pass2(tiles):
                nonlocal p2_done
                f0 = c0(tiles[0])
                f1 = c0(tiles[-1]) + nch(tiles[-1])
                for f in range(f0, f1):
                    for h in range(2):
                        nc.tensor.matmul(
                            cps[h][:, :],
                            attn2b[:, f, :],
                            en2_sb[:, f, h * 512:(h + 1) * 512],
                            start=(p2_done == 0),
                            stop=False,
                        )
                    p2_done += 1

            last_gi = len(groups) - 1
            for gi, tiles in enumerate(groups):
                accs = {}
                en7 = {} if gi == last_gi else None
                for ec in range(DC):
                    pss = {
                        t_: psA.tile([128, ST], f32, tag="proj", name=f"ps{t_}_{ec}")
                        for t_ in tiles
                    }
                    for t_ in tiles:
                        w_ = nch(t_) * 128
                        if t_ == 0:
                            # bf16 repair tile: full-precision scores for
                            # the short batches packed at the front.
                            for kc in range(DC):
                                nc.tensor.matmul(
                                    pss[t_][:, 0:w_],
                                    weTsB_sb[:, ec, kc, :],
                                    etb_sb[:, kc, 0:w_],
                                    start=(kc == 0),
                                    stop=(kc == DC - 1),
                                )
                        else:
                            # fp8 DoubleRow: each matmul contracts a PAIR of
                            # adjacent 128-chunks (K=256) at half bf16 cost.
                            for kc in range(DC // 2):
                                nc.tensor.matmul(
                                    pss[t_][:, 0:w_],
                                    weTs8_sb[:, ec, 2 * kc:2 * kc + 2, :],
                                    et8_sb[:, t_ - 1, 2 * kc:2 * kc + 2, 0:w_],
                                    start=(kc == 0),
                                    stop=(kc == DC // 2 - 1),
                                    perf_mode=DoubleRow,
                                )
                    # deferred post-work of the previous group, staged so
                    # the PE queue has matmul runway ahead of the deps.
                    if pend is not None:
                        if ec == 3:
                            pend_sps = emit_reduces(*pend)
                        elif ec == 5:
                            emit_softmax(pend[0], pend_sps)
                        elif ec == 6:
                            emit_pass2(pend[0])
                            pend = None
                    for t_ in tiles:
                        w_ = nch(t_) * 128
                        if en7 is not None and ec == DC - 1:
                            # tail group, last ec: chunked tanh; its v-dot
                            # is folded into the reduce matmuls.
                            e7 = enp.tile([128, ST], bf16, tag="en7")
                            en7[t_] = e7
                            cb = c0(t_)
                            for j in range(nch(t_)):
                                nc.scalar.activation(
                                    e7[:, j * 128:(j + 1) * 128],
                                    pss[t_][:, j * 128:(j + 1) * 128],
                                    Tanh,
                                    bias=biasC_sb[:, ec, cb + j:cb + j + 1],
                                )
                            continue
                        en = enp.tile([128, ST], bf16, tag="en", bufs=6)
                        tanh_emit(en, pss[t_], ec, t_, w_)
                        if ec == 0:
                            acc = enp.tile([128, ST], bf16, tag="acc", bufs=7)
                            accs[t_] = acc
                            nc.vector.tensor_scalar_mul(
                                acc[:, 0:w_], en[:, 0:w_], vcol_sb[:, 0:1]
                            )
                        else:
                            nc.vector.scalar_tensor_tensor(
                                accs[t_][:, 0:w_], en[:, 0:w_],
                                vcol_sb[:, ec:ec + 1], accs[t_][:, 0:w_],
                                op0=Alu.mult, op1=Alu.add,
                            )
                pend = (tiles, accs, en7)

            # tail: post-work of the last group
            pend_sps = emit_reduces(*pend)
            emit_softmax(pend[0], pend_sps)
            if stage == "p1":
                nc.gpsimd.dma_start(out=out_d[:, :], in_=scores_sb)
            elif stage == "sm":
                nc.gpsimd.dma_start(out=out_d[:, :], in_=exp_sb)
            else:
                # denominators on the DVE right behind the attn2 builds; the
                # totals matmul slots between the h0 and h1 pass-2 blocks.
                nc.vector.scalar_tensor_tensor(
                    mexp0, exp_sb, 1.0, mask0, op0=Alu.mult, op1=Alu.mult,
                    accum_out=psums01[:, 0:1],
                )
                nc.vector.scalar_tensor_tensor(
                    mexp1, exp_sb, 1.0, mask1, op0=Alu.mult, op1=Alu.mult,
                    accum_out=psums01[:, 1:2],
                )
                rinv2 = consts.tile([BL, 1], f32)
                pst = psS.tile([BL, 1], f32, tag="s", name="pst")
                f0 = c0(pend[0][0])
                f1 = c0(pend[0][-1]) + nch(pend[0][-1])
                for f in range(f0, f1):
                    nc.tensor.matmul(
                        cps[0][:, :], attn2b[:, f, :],
                        en2_sb[:, f, 0:512],
                        start=(p2_done == 0 and f == f0), stop=(f == f1 - 1),
                    )
                nc.tensor.matmul(pst, psums01, ones32[:, 0:1], start=True, stop=True)
                for f in range(f0, f1):
                    nc.tensor.matmul(
                        cps[1][:, :], attn2b[:, f, :],
                        en2_sb[:, f, 512:1024],
                        start=(p2_done == 0 and f == f0), stop=(f == f1 - 1),
                    )
                p2_done += f1 - f0
                assert p2_done == nf, (p2_done, nf)
                nc.vector.reciprocal(rinv2, pst)
                ctx0 = consts.tile([BL, 512], f32)
                nc.vector.tensor_scalar_mul(ctx0, cps[0], rinv2)
                nc.sync.dma_start(out=out_d[:, 0:512], in_=ctx0)
                ctx1 = consts.tile([BL, 512], f32)
                nc.scalar.mul(ctx1, cps[1], rinv2)
                nc.gpsimd.dma_start(out=out_d[:, 512:1024], in_=ctx1)

    nc.compile()
    return nc


def _get_nc(nt, nf, mz, mzlo, stage="all"):
    key = (nt, nf, mz, mzlo, stage)
    if key not in _NC_CACHE:
        _NC_CACHE[key] = _build_program(nt, nf, mz, mzlo, stage)
    return _NC_CACHE[key]


def _plan(lengths):
    """Pair batches longest-with-shortest; the SHORT batch packs first
    (into the bf16 repair tile), the long one right behind at 128-chunk
    granularity.  NF is the max over cores; mz is the watermark below
    which chunk ownership varies per core."""
    l = np.asarray(lengths, dtype=np.int64)
    c128 = (np.clip(l, 1, S) + 127) // 128
    order = np.argsort(-c128, kind="stable")
    pairs = [(int(order[B - 1 - i]), int(order[i])) for i in range(NCORES)]
    nf = int(max(c128[s] + c128[g] for s, g in pairs))
    nf = max(nf, REP0 + 2 * NPT + 1)   # keep the group structure (>= 4 tiles)
    nt = 1 + (nf - REP0 + NPT - 1) // NPT
    mz = int(max(c128[s] for s, _ in pairs))
    mzlo = int(min(c128[s] for s, _ in pairs))
    return pairs, c128, nt, nf, mz, mzlo


def _make_in_maps(encoder_outputs, hidden, lengths, W, b, v):
    import ml_dtypes

    bf16 = ml_dtypes.bfloat16
    f8 = ml_dtypes.float8_e4m3
    enc = np.asarray(encoder_outputs, dtype=np.float32)
    hid = np.asarray(hidden, dtype=np.float32)
    len_ = np.asarray(lengths, dtype=np.int64)
    Wn = np.asarray(W, dtype=np.float32)
    bn = np.asarray(b, dtype=np.float32)
    vn = np.asarray(v, dtype=np.float32)

    pairs, c128, nt, nf, mz, mzlo = _plan(len_)

    # per-ec weight stripes, partition-major:
    # w[p, ec, kc, q] = We.T[kc*128+p, ec*128+q]
    weT = Wn[:, D:].T.reshape(DC, 128, DC, 128).transpose(1, 2, 0, 3)
    weTsB = np.ascontiguousarray(weT.astype(bf16))
    weTs8 = np.ascontiguousarray(weT.astype(f8))
    vcol = np.ascontiguousarray(vn.reshape(DC, 128).T)
    vcolb = vcol.astype(bf16)
    # hid bias on host: bias_x = hid[x] @ Wh.T + b  (trivial vs pass-1)
    biasH = hid @ Wn[:, :D].T + bn            # [B, D]

    in_maps = []
    r0 = REP0 * 128
    for s_, g_ in pairs:
        ns, ng = int(c128[s_]), int(c128[g_])
        packed = np.zeros((r0 + (nt - 1) * ST, D), dtype=np.float32)
        packed[:ns * 128] = enc[s_, :ns * 128]
        packed[ns * 128:(ns + ng) * 128] = enc[g_, :ng * 128]
        etb = np.ascontiguousarray(
            packed[:r0].reshape(r0, DC, 128).transpose(2, 1, 0).astype(bf16)
        )
        et8 = np.zeros((nt - 1, 128, DC * ST), dtype=f8)
        for t_ in range(1, nt):
            w_ = min(NPT * 128, nf * 128 - r0 - (t_ - 1) * ST)
            blk = packed[r0 + (t_ - 1) * ST:r0 + (t_ - 1) * ST + w_]
            et8[t_ - 1, :, :DC * w_] = (
                blk.reshape(w_, DC, 128).transpose(2, 1, 0).reshape(128, DC * w_)
            ).astype(f8)
        encf = np.ascontiguousarray(
            packed.astype(bf16).reshape(REP0 + (nt - 1) * NPT, 128, D)[:nf]
            .transpose(1, 0, 2)
        )

        biasC = np.empty((128, DC, nf), dtype=np.float32)
        posf = np.full((128, nf), 1.0e9, dtype=np.float32)
        lenmap = np.zeros((128, nf), dtype=np.float32)
        own0 = np.zeros((128, nf), dtype=np.float32)
        p = np.arange(128, dtype=np.float32)
        bias_s = biasH[s_].reshape(DC, 128).T     # [128, DC]
        bias_g = biasH[g_].reshape(DC, 128).T
        for f in range(nf):
            if f < ns:
                biasC[:, :, f] = bias_s
                posf[:, f] = f * 128 + p
                lenmap[:, f] = float(len_[s_])
                own0[:, f] = 1.0
            else:
                biasC[:, :, f] = bias_g
                if f < ns + ng:
                    posf[:, f] = (f - ns) * 128 + p
                    lenmap[:, f] = float(len_[g_])

        pmo = np.ascontiguousarray(np.stack([posf, lenmap, own0], axis=1))
        in_maps.append(
            dict(
                etb=etb, et8=et8, encf=encf,
                weTsB=weTsB, weTs8=weTs8, biasC=np.ascontiguousarray(biasC),
                vcol=vcol, vcolb=vcolb, pmo=pmo,
            )
        )
    return in_maps, pairs, nt, nf, mz, mzlo


def run(inputs, trace=False, stage="all"):
    """Run on 8 NeuronCores; returns (output [B,1,D], BassKernelResults)."""
    from concourse.bass_utils import run_bass_kernel_spmd

    in_maps, pairs, nt, nf, mz, mzlo = _make_in_maps(**inputs)
    nc = _get_nc(nt, nf, mz, mzlo, stage)
    r = run_bass_kernel_spmd(
        nc, in_maps, core_ids=list(range(NCORES)), trace=trace
    )
    if stage != "all":
        out = np.stack(
            [np.asarray(r.results[i]["ctx_out"]) for i in range(NCORES)], axis=0
        )
        return out, r, pairs
    out = np.empty((B, 1, D), dtype=np.float32)
    for i, (s_, g_) in enumerate(pairs):
        ctx = np.asarray(r.results[i]["ctx_out"])
        out[s_, 0] = ctx[0]
        out[g_, 0] = ctx[1]
    return out, r


def kernel(encoder_outputs, hidden, lengths, W, b, v):
    out, _ = run(
        dict(
            encoder_outputs=encoder_outputs,
            hidden=hidden,
            lengths=lengths,
            W=W,
            b=b,
            v=v,
        )
    )
    return out
